# revision 1
# baseline (speedup 1.0000x reference)
"""DeepState (2-layer GRU + linear SSM head) Trainium2 kernel.

Strategy:
  - 8-way data parallel over batch (B=256 -> 32 per core), SPMD.
  - Per core: layer-0 GRU (512 steps), layer-1 GRU (512 steps), then one
    GEMM that folds the projection + the 96-step linear SSM scan (the scan
    matrix powers are input-only, so they're precomputed on host and folded
    into the projection weight).
  - Hidden state kept transposed on-chip: [128 partitions = hidden-chunk,
    free = chunk*B + b].
  - Per 16-step chunk: the input-projection GEMM runs into its own PSUM
    banks, then per-gate-block ACT copies move it to SBUF folding the
    biases in.  Each recurrence step's W_hh matmuls write one PSUM bank.
"""

import sys

for _p in ("/opt/trn_rl_repo",):
    if _p not in sys.path:
        sys.path.insert(0, _p)

import numpy as np

# ---------------------------------------------------------------- constants
N_CORES = 8
B_FULL = 256
S_FULL = 512
IN = 32
H = 256
G = 3 * H          # 768 gate rows
NB = H // 128      # 2 hidden chunks
D = 32
STATE = 4
PRED = 96
TD = PRED * D      # 3072 tail output rows
B = B_FULL // N_CORES  # 32 per core
CH = 8             # steps per chunk (layer-1 pipelined one chunk behind)


def _imports():
    from concourse import bacc, bass, mybir
    from concourse.tile import TileContext
    return bacc, bass, mybir, TileContext


# ---------------------------------------------------------------- builder
def build_kernel(S=S_FULL, ch=CH):
    """Build the SPMD bass program (same for every core).

    Layers are software-pipelined: layer 1 runs one chunk (ch steps) behind
    layer 0, so the two per-step dependency chains overlap on the engines
    and layer 0's hidden states feed layer 1's input GEMM straight from
    SBUF (no DRAM round trip).
    """
    bacc, bass, mybir, TileContext = _imports()
    f32 = mybir.dt.float32
    ALU = mybir.AluOpType
    ACTF = mybir.ActivationFunctionType

    assert S % ch == 0
    NCH = S // ch

    nc = bacc.Bacc(None, target_bir_lowering=False)

    # -------- dram parameters (per-core shapes)
    xT = nc.declare_dram_parameter("xT", [S, IN, B], mybir.dt.float16,
                                   isOutput=False)
    w0T = nc.declare_dram_parameter("w0T", [IN, G], mybir.dt.float16,
                                    isOutput=False)
    bf16 = mybir.dt.float16  # 16-bit matmul dtype (fp16: 10-bit mantissa)
    whh0T = nc.declare_dram_parameter("whh0T", [H, G], bf16, isOutput=False)
    w1T = nc.declare_dram_parameter("w1T", [H, G], bf16, isOutput=False)
    whh1T = nc.declare_dram_parameter("whh1T", [H, G], bf16, isOutput=False)
    # per layer: 6 bias columns (j=0..3 r/z: b_ih+b_hh ; j=4,5 n: b_ih)
    brzxn = nc.declare_dram_parameter("brzxn", [128, 2 * 6], f32, isOutput=False)
    # per layer: n-gate b_hh replicated over batch: [128, layer*NB*B + cc*B + b]
    bhhn = nc.declare_dram_parameter(
        "bhhn", [128, 2 * NB * B], f32, isOutput=False
    )
    ident = nc.declare_dram_parameter("ident", [128, 128], mybir.dt.float16,
                                      isOutput=False)
    wbigT = nc.declare_dram_parameter("wbigT", [H, TD], f32, isOutput=False)
    bbig = nc.declare_dram_parameter("bbig", [128, TD // 128], f32, isOutput=False)
    yT = nc.declare_dram_parameter("yT", [TD, B], f32, isOutput=True)

    MT = TD // 128  # 24 tail m-tiles
    CB = ch * B     # tokens per chunk

    with TileContext(nc) as tc:
        with (
            tc.tile_pool(name="wres", bufs=1) as wres,
            tc.tile_pool(name="bres", bufs=1) as bres,
        ):
            # resident weights
            w0_sb = wres.tile([IN, G], bf16, name="w0_sb")
            nc.sync.dma_start(out=w0_sb[:], in_=w0T[:])
            whh_sb = []  # [layer] -> [128, NB*G]
            for li, src_t in enumerate((whh0T, whh1T)):
                t = wres.tile([128, NB * G], bf16, name=f"whh{li}_sb")
                for kc in range(NB):
                    nc.sync.dma_start(
                        out=t[:, kc * G:(kc + 1) * G],
                        in_=src_t[kc * 128:(kc + 1) * 128, :],
                    )
                whh_sb.append(t)
            w1_sb = wres.tile([128, NB * G], bf16, name="w1_sb")
            for kc in range(NB):
                nc.sync.dma_start(
                    out=w1_sb[:, kc * G:(kc + 1) * G],
                    in_=w1T[kc * 128:(kc + 1) * 128, :],
                )
            ident_sb = wres.tile([128, 128], bf16, name="ident_sb")
            nc.sync.dma_start(out=ident_sb[:], in_=ident[:])
            brzxn_sb = bres.tile([128, 2 * 6], f32, name="brzxn_sb")
            nc.sync.dma_start(out=brzxn_sb[:], in_=brzxn[:])
            bhhn_sb = bres.tile([128, 2 * NB * B], f32, name="bhhn_sb")
            nc.sync.dma_start(out=bhhn_sb[:], in_=bhhn[:])
            bbig_sb = bres.tile([128, MT], f32, name="bbig_sb")
            nc.sync.dma_start(out=bbig_sb[:], in_=bbig[:])
            # tail-GEMM weights last: nothing needs them until the end, so
            # they must not delay the first chunk's x/weight loads
            wbig_sb = wres.tile([128, NB * TD], f32, name="wbig_sb")
            with tc.high_priority(offset=-10000):
                pass
            for kc in range(NB):
                nc.sync.dma_start(
                    out=wbig_sb[:, kc * TD:(kc + 1) * TD],
                    in_=wbigT[kc * 128:(kc + 1) * 128, :],
                )

            with (
                tc.tile_pool(name="xt", bufs=3) as xt_pool,
                tc.tile_pool(name="ring", bufs=3) as ring_pool,
                tc.tile_pool(name="psx0", bufs=1, space="PSUM") as psx0_pool,
                tc.tile_pool(name="psx1", bufs=1, space="PSUM") as psx1_pool,
                tc.tile_pool(name="ghp", bufs=2, space="PSUM") as gh_pool,
                tc.tile_pool(name="xps0", bufs=2) as xp0_sb_pool,
                tc.tile_pool(name="xps1", bufs=2) as xp1_sb_pool,
                tc.tile_pool(name="h2p", bufs=3) as h2_pool,
                tc.tile_pool(name="work", bufs=6) as work,
            ):
                def xp_thunks(layer, psx_pool, xp_sb_pool, lhs_sb,
                              rhs_aps, nk):
                    """Thunk list for a chunk input-projection GEMM + biased
                    copies to SBUF.  Emitted diffused between recurrence
                    steps so they never form a belt in an engine queue."""
                    psx = psx_pool.tile([128, 6 * CB], f32, tag="psx")
                    # r/z blocks as fp16 (identity-matmul operand), n blocks
                    # as fp32 (read by the n_arg elementwise op)
                    xp_sb = xp_sb_pool.tile([128, 4 * CB], bf16, tag="xp16")
                    xpn_sb = xp_sb_pool.tile([128, 2 * CB], f32, tag="xpn")
                    thunks = []

                    def mk_mm(j, kc):
                        def go():
                            nc.tensor.matmul(
                                psx[:, j * CB:(j + 1) * CB],
                                lhs_sb[kc][:, j * 128:(j + 1) * 128],
                                rhs_aps[kc],
                                start=(j % 2 == 0 and kc == 0),
                                stop=(j % 2 == 1 and kc == nk - 1),
                            )
                        return go

                    def mk_cp(j, half):
                        # half-block copies: shorter FIFO occupancy so chain
                        # ops slot between them
                        HB = CB // 2
                        def go_inner():
                            off = half * HB
                            if j >= 4:
                                nc.vector.tensor_scalar_add(
                                    xpn_sb[:, (j - 4) * CB + off:
                                           (j - 4) * CB + off + HB],
                                    psx[:, j * CB + off:j * CB + off + HB],
                                    brzxn_sb[:, layer * 6 + j:
                                             layer * 6 + j + 1],
                                )
                            else:
                                nc.vector.tensor_scalar_add(
                                    xp_sb[:, j * CB + off:j * CB + off + HB],
                                    psx[:, j * CB + off:j * CB + off + HB],
                                    brzxn_sb[:, layer * 6 + j:
                                             layer * 6 + j + 1],
                                )

                        def go():
                            with tc.high_priority(offset=-60):
                                go_inner()
                        return go

                    # bank-pair order so each bank's group closes before its
                    # copies run
                    for jp in range(3):
                        for j in (2 * jp, 2 * jp + 1):
                            for kc in range(nk):
                                thunks.append(mk_mm(j, kc))
                        for j in (2 * jp, 2 * jp + 1):
                            for half in range(2):
                                thunks.append(mk_cp(j, half))
                    return (xp_sb, xpn_sb), thunks

                def gru_step(layer, h_prev_kc, xps, tl, hnew_view3, hnew_kc):
                    """One GRU step.  h_prev_kc: per-chunk [128,B] APs of the
                    previous hidden state; hnew_view3: [128, NB, B] AP to
                    write the new state; hnew_kc: same as per-chunk APs."""
                    xp_sb, xpn_sb = xps
                    whh_l = whh_sb[layer]
                    ghp = gh_pool.tile([128, 6 * B], f32, tag="ghp")
                    # inject this step's r/z x-projection into the bank via
                    # identity matmuls (PE does the add, no DVE pass needed);
                    # these only depend on xp so they run ahead of the chain
                    for j in range(4):
                        nc.tensor.matmul(
                            ghp[:, j * B:(j + 1) * B],
                            ident_sb[:],
                            xp_sb[:, j * CB + tl * B:j * CB + (tl + 1) * B],
                            start=(j == 0),
                            stop=False,
                        )
                    for ji, j in enumerate(range(6)):
                        for kc in range(NB):
                            nc.tensor.matmul(
                                ghp[:, j * B:(j + 1) * B],
                                whh_l[:, kc * G + j * 128:
                                      kc * G + (j + 1) * 128],
                                h_prev_kc[kc],
                                start=False,
                                stop=(ji == 5 and kc == NB - 1),
                            )

                    rz = work.tile([128, 4 * B], bf16, tag=f"rz{layer}")
                    nc.scalar.activation(
                        rz[:], ghp[:, 0:4 * B], ACTF.Sigmoid
                    )
                    hn = work.tile([128, NB * B], bf16, tag=f"hn{layer}")
                    nc.vector.tensor_add(
                        hn[:], ghp[:, 4 * B:6 * B],
                        bhhn_sb[:, layer * NB * B:(layer + 1) * NB * B],
                    )

                    # n-gate chain first on GPSIMD (zh/omz are only needed
                    # after tanh, so they go behind prod/n_arg in the FIFO)
                    prod = work.tile([128, NB * B], f32, tag=f"prod{layer}")
                    nc.gpsimd.tensor_mul(prod[:], rz[:, 0:NB * B], hn[:])
                    n_arg = work.tile([128, NB * B], f32, tag=f"narg{layer}")
                    nc.gpsimd.tensor_add(
                        n_arg[:].rearrange("p (j b) -> p j b", b=B),
                        prod[:].rearrange("p (j b) -> p j b", b=B),
                        xpn_sb[:].rearrange("p (j b) -> p j b", b=CB)
                        [:, 0:NB, tl * B:(tl + 1) * B],
                    )
                    zh = work.tile([128, NB * B], f32, tag=f"zh{layer}")
                    zv = rz[:, NB * B:2 * NB * B]
                    for kc in range(NB):
                        nc.gpsimd.tensor_mul(
                            zh[:, kc * B:(kc + 1) * B],
                            zv[:, kc * B:(kc + 1) * B],
                            h_prev_kc[kc],
                        )
                    omz = work.tile([128, NB * B], f32, tag=f"omz{layer}")
                    nc.gpsimd.tensor_scalar(
                        omz[:], zv, -1.0, 1.0, op0=ALU.mult, op1=ALU.add
                    )
                    n_t = work.tile([128, NB * B], f32, tag=f"nt{layer}")
                    nc.scalar.activation(n_t[:], n_arg[:], ACTF.Tanh)

                    f_t = work.tile([128, NB * B], f32, tag=f"ft{layer}")
                    nc.gpsimd.tensor_mul(f_t[:], n_t[:], omz[:])
                    nc.gpsimd.tensor_add(
                        hnew_view3,
                        f_t[:].rearrange("p (j b) -> p j b", b=B),
                        zh[:].rearrange("p (j b) -> p j b", b=B),
                    )

                # initial states
                h0z = work.tile([128, NB * B], bf16, name="h0z", bufs=1)
                nc.gpsimd.memset(h0z[:], 0.0)
                h0_kc = [h0z[:, kc * B:(kc + 1) * B] for kc in range(NB)]
                h2z = work.tile([128, NB * B], bf16, name="h2z", bufs=1)
                nc.gpsimd.memset(h2z[:], 0.0)
                h2_kc = [h2z[:, kc * B:(kc + 1) * B] for kc in range(NB)]

                LAG = 2  # layer 1 runs two chunks behind layer 0
                xp0_tiles = {}
                xp1_tiles = {}
                rings = {}
                h2_final = None
                from collections import deque
                pending = deque()

                def load_xt(c):
                    xt_sb = xt_pool.tile([IN, CB], bf16, tag="xt")
                    nc.sync.dma_start(
                        out=xt_sb[:].rearrange("r (t b) -> r t b", t=ch),
                        in_=xT[c * ch:(c + 1) * ch].rearrange("t r b -> r t b"),
                    )
                    return xt_sb

                def gen_work(c):
                    """Generate diffused thunks at the start of superchunk c:
                    xp0 for chunk c+1, xp1 over ring(c-1)."""
                    th = []
                    if c + 1 < NCH:
                        xt_sb = load_xt(c + 1)
                        xp0_tiles[c + 1], t0 = xp_thunks(
                            0, psx0_pool, xp0_sb_pool, [w0_sb], [xt_sb[:]], 1)
                        th += t0
                    if 1 <= c <= NCH:
                        rprev = rings.pop(c - 1)
                        xp1_tiles[c - 1], t1 = xp_thunks(
                            1, psx1_pool, xp1_sb_pool,
                            [w1_sb[:, kc * G:(kc + 1) * G] for kc in range(NB)],
                            [rprev[:, kc * CB:(kc + 1) * CB]
                             for kc in range(NB)], NB)
                        th += t1
                    return th

                # prologue: chunk 0's xp emitted directly
                xt0 = load_xt(0)
                xp0_tiles[0], t_pro = xp_thunks(
                    0, psx0_pool, xp0_sb_pool, [w0_sb], [xt0[:]], 1)
                for t in t_pro:
                    t()

                for c in range(NCH + LAG):
                    pending.extend(gen_work(c))
                    c1 = c - LAG  # token chunk L1 is working on
                    per_step = (len(pending) + ch - 1) // ch if pending else 0
                    if c < NCH:
                        ring = ring_pool.tile([128, NB * CB], bf16, tag="ring")
                        rings[c] = ring
                    half_step = (per_step + 1) // 2
                    for tl in range(ch):
                        # emit diffused xp work first: it lands in the engine
                        # FIFOs *before* this step's chain ops, so it fills
                        # the wait for the previous step's h_new instead of
                        # blocking the new one
                        for _ in range(half_step):
                            if pending:
                                pending.popleft()()
                        if c < NCH:
                            rv = ring[:].rearrange(
                                "p (k tb) -> p k tb", tb=CB
                            )[:, :, tl * B:(tl + 1) * B]
                            rkc = [ring[:, kc * CB + tl * B:
                                        kc * CB + (tl + 1) * B]
                                   for kc in range(NB)]
                            gru_step(0, h0_kc, xp0_tiles[c], tl, rv, rkc)
                            h0_kc = rkc
                        if c1 >= 0:
                            for _ in range(half_step):
                                if pending:
                                    pending.popleft()()
                            h2n = h2_pool.tile([128, NB * B], bf16, tag="h2")
                            nkc = [h2n[:, kc * B:(kc + 1) * B]
                                   for kc in range(NB)]
                            gru_step(
                                1, h2_kc, xp1_tiles[c1], tl,
                                h2n[:].rearrange("p (k b) -> p k b", b=B),
                                nkc,
                            )
                            h2_kc = nkc
                            if c1 == NCH - 1 and tl == ch - 1:
                                h2_final = h2n
                    while pending:
                        pending.popleft()()

                # copy final hidden state to a persistent tile so the tail
                # can use it after the recurrence pools close
                h_final = bres.tile([128, NB * B], f32, name="h_final")
                nc.vector.tensor_copy(h_final[:], h2_final[:])

            # ---- tail: y = Wbig @ h2 + bbig
            with (
                tc.tile_pool(name="tailp", bufs=1, space="PSUM") as tailp,
                tc.tile_pool(name="yout", bufs=4) as yout,
            ):
                ps = tailp.tile([128, MT * B], f32)  # 24*32 = 768 cols
                PER_BANK = 512 // B
                for mt in range(MT):
                    for kc in range(NB):
                        nc.tensor.matmul(
                            ps[:, mt * B:(mt + 1) * B],
                            wbig_sb[:, kc * TD + mt * 128:
                                    kc * TD + (mt + 1) * 128],
                            h_final[:, kc * B:(kc + 1) * B],
                            start=(kc == 0 and mt % PER_BANK == 0),
                            stop=(kc == NB - 1 and
                                  (mt % PER_BANK == PER_BANK - 1
                                   or mt == MT - 1)),
                        )
                for mt in range(MT):
                    yt = yout.tile([128, B], f32, tag="yt")
                    nc.vector.tensor_scalar_add(
                        yt[:], ps[:, mt * B:(mt + 1) * B],
                        bbig_sb[:, mt:mt + 1],
                    )
                    nc.sync.dma_start(
                        out=yT[mt * 128:(mt + 1) * 128, :], in_=yt[:]
                    )

    nc.finalize()
    return nc


# ---------------------------------------------------------------- host prep
def prep_core_inputs(inputs, S=S_FULL):
    """Build per-core input maps from the full problem inputs."""
    x = np.asarray(inputs["x"], np.float32)[:, :S]
    W_ih_l0 = np.asarray(inputs["W_ih_l0"], np.float32)
    W_hh_l0 = np.asarray(inputs["W_hh_l0"], np.float32)
    b_ih_l0 = np.asarray(inputs["b_ih_l0"], np.float32)
    b_hh_l0 = np.asarray(inputs["b_hh_l0"], np.float32)
    W_ih_l1 = np.asarray(inputs["W_ih_l1"], np.float32)
    W_hh_l1 = np.asarray(inputs["W_hh_l1"], np.float32)
    b_ih_l1 = np.asarray(inputs["b_ih_l1"], np.float32)
    b_hh_l1 = np.asarray(inputs["b_hh_l1"], np.float32)
    W_proj = np.asarray(inputs["W_proj"], np.float32)
    b_proj = np.asarray(inputs["b_proj"], np.float32)
    C = np.asarray(inputs["C"], np.float32)
    rld = np.asarray(inputs["raw_level_decay"], np.float32)
    rtd = np.asarray(inputs["raw_trend_decay"], np.float32)
    rg = np.asarray(inputs["raw_gamma"], np.float32)
    omega = np.asarray(inputs["omega"], np.float32)

    def sig(v):
        return 1.0 / (1.0 + np.exp(-v.astype(np.float64)))

    # --- fold the SSM scan into the projection
    a_l = sig(rld) * 0.15 + 0.85
    a_t = sig(rtd) * 0.25 + 0.7
    g = sig(rg) * 0.2 + 0.8
    cw, sw = np.cos(omega.astype(np.float64)), np.sin(omega.astype(np.float64))
    T = np.zeros((D, STATE, STATE), np.float64)
    T[:, 0, 0] = a_l
    T[:, 1, 1] = a_t
    # new2 = s2*rot00 + s3*rot10 ; new3 = s2*rot01 + s3*rot11
    T[:, 2, 2] = g * cw
    T[:, 2, 3] = g * sw
    T[:, 3, 2] = -g * sw
    T[:, 3, 3] = g * cw
    K = np.zeros((PRED, D, STATE), np.float64)
    cur = np.einsum("ds,dsj->dj", C.astype(np.float64), T)  # C @ T
    K[0] = cur
    for i in range(1, PRED):
        cur = np.einsum("dj,djk->dk", cur, T)
        K[i] = cur
    Wp = W_proj.astype(np.float64).reshape(D, STATE, H)
    bp = b_proj.astype(np.float64).reshape(D, STATE)
    Wbig = np.einsum("tdj,djh->tdh", K, Wp).reshape(TD, H)
    bbig_vec = np.einsum("tdj,dj->td", K, bp).reshape(TD)
    wbigT = np.ascontiguousarray(Wbig.T.astype(np.float32))
    bbig = np.ascontiguousarray(
        bbig_vec.reshape(TD // 128, 128).T.astype(np.float32)
    )

    import ml_dtypes
    bf = np.float16
    w0T = np.ascontiguousarray(W_ih_l0.T).astype(np.float16)
    whh0T = np.ascontiguousarray(W_hh_l0.T).astype(bf)
    whh1T = np.ascontiguousarray(W_hh_l1.T).astype(bf)
    w1T = np.ascontiguousarray(W_ih_l1.T).astype(bf)

    # bias columns [128, 2 layers * 6 blocks]
    brzxn = np.zeros((128, 12), np.float32)
    bhhn = np.zeros((128, 2 * NB * B), np.float32)
    for li, (bi, bh) in enumerate(((b_ih_l0, b_hh_l0), (b_ih_l1, b_hh_l1))):
        full = bi.copy()
        full[: 2 * H] += bh[: 2 * H]
        for j in range(6):
            brzxn[:, li * 6 + j] = full[j * 128:(j + 1) * 128]
        for cc in range(NB):
            col = bh[2 * H + cc * 128: 2 * H + (cc + 1) * 128]
            bhhn[:, (li * NB + cc) * B:(li * NB + cc + 1) * B] = col[:, None]

    shared = dict(
        w0T=w0T, whh0T=whh0T, w1T=w1T, whh1T=whh1T,
        brzxn=brzxn, bhhn=bhhn, wbigT=wbigT, bbig=bbig,
        ident=np.eye(128, dtype=np.float16),
    )
    maps = []
    for i in range(N_CORES):
        xs = x[i * B:(i + 1) * B]  # [B, S, IN]
        m = dict(shared)
        m["xT"] = np.ascontiguousarray(
            xs.transpose(1, 2, 0).astype(np.float16))
        maps.append(m)
    return maps


def assemble_output(results):
    """results: list of per-core dicts with 'yT' [TD, B] -> full [256,96,32]."""
    y = np.empty((B_FULL, PRED, D), np.float32)
    for i, r in enumerate(results):
        y[i * B:(i + 1) * B] = r["yT"].reshape(PRED, D, B).transpose(2, 0, 1)
    return y


# ---------------------------------------------------------------- entry point
_CACHE = {}


def _get_nc(S=S_FULL):
    if S not in _CACHE:
        _CACHE[S] = build_kernel(S)
    return _CACHE[S]


def kernel(**inputs):
    from concourse.bass_utils import run_bass_kernel_spmd

    nc = _get_nc(S_FULL)
    maps = prep_core_inputs(inputs, S_FULL)
    res = run_bass_kernel_spmd(nc, maps, list(range(N_CORES)))
    return assemble_output(res.results)

